# revision 7
# baseline (speedup 1.0000x reference)
"""Trainium2 Bass kernel for the Dominant GNN autoencoder.

Model (reference semantics):
    5 GCN convs (shared encoder 512->128->128, attribute decoder
    128->128->512, structure decoder 128->128) over a fixed graph with
    symmetric gcn_norm and self-loops, then a_hat = s @ s.T and x_hat.

Strategy (8 NeuronCores):
  - Nodes are sharded row-wise: core c owns nodes [c*1536, (c+1)*1536).
  - Edges are routed (on host) to the core owning their *destination*,
    sorted by (dst tile, src), and packed into fixed-size blocks of 128
    edge slots.  For each 128-slot block a one-hot "selector" matrix
    [128 slots, 128 dst lanes] holding the raw edge weight w_e is
    prebuilt on the host (pure layout transform of edge_index/
    edge_weight; no model math is done on the host).
  - On device, a GCN aggregation is:
        u = h @ W            (per-core rows)
        g = dinv * u         (per-partition scale; dinv = deg^-1/2)
        g_full = AllGather(g)
        G = dma_gather(g_full, src-index per edge slot)
        P = sum_blocks Sel_b.T @ G_b        (PE, PSUM accumulate)
          + sqrt(deg) x bias               (K=1 matmul folds the bias)
        h' = Relu(dinv * P)                (ACT, per-partition scale)
    which equals segment_sum(norm * (h@W)[src], dst) + b with
    norm = dinv[src] * w * dinv[dst].
  - deg itself is computed on device from the selector blocks
    (deg = Sel.T @ 1).
  - a_hat rows are sharded: each core computes s_own @ s_full.T after an
    AllGather of s, and writes its [1536, 12288] f32 slice.
  - The last conv (128->512) is commuted to aggregate-then-linear so all
    aggregations run in 128-wide feature space.

Compute dtype is bf16 on the PE (f32 PSUM accumulation); outputs are f32.
"""

import math

import ml_dtypes
import numpy as np

import concourse.bacc as bacc
import concourse.bass as bass
import concourse.mybir as mybir
import concourse.tile as tile
from concourse import library_config
from concourse.bass_utils import run_bass_kernel_spmd

BF16 = mybir.dt.bfloat16
F32 = mybir.dt.float32
I16 = mybir.dt.int16

C = 8          # cores
P = 128        # partitions
GC = 6         # gather chunk, in 128-slot blocks

# full-size problem dims (hardcoded per harness contract)
N_FULL, FEAT_FULL, HID_FULL = 12288, 512, 128


def _prep(x, edge_index, edge_weight, n, hid):
    """Route/sort/pack edges per core. Returns per-core host arrays + meta."""
    nl = n // C           # nodes per core
    t_per_core = nl // P  # dst tiles per core
    n_tiles = n // P

    src = np.concatenate([edge_index[0], np.arange(n)]).astype(np.int64)
    dst = np.concatenate([edge_index[1], np.arange(n)]).astype(np.int64)
    w = np.concatenate(
        [np.asarray(edge_weight, np.float32), np.ones(n, np.float32)]
    )

    gtile = dst // P
    order = np.lexsort((src, gtile))
    src, dst, w, gtile = src[order], dst[order], w[order], gtile[order]

    counts = np.bincount(gtile, minlength=n_tiles)
    bhat = GC * int(math.ceil(counts.max() / (P * GC)))  # blocks per tile
    slots_per_tile = bhat * P
    slots_per_core = t_per_core * slots_per_tile

    offs = np.concatenate([[0], np.cumsum(counts)])

    sel = np.zeros((C, P, slots_per_core), ml_dtypes.bfloat16)
    gidx = np.zeros((C, 16, slots_per_core // 16), np.int16)

    for gt in range(n_tiles):
        c, t = divmod(gt, t_per_core)
        o0, o1 = offs[gt], offs[gt + 1]
        k = o1 - o0
        s = np.arange(k)
        # selector: [slot%128 (partition), (t*bhat + slot//128)*128 + dstlane]
        col = (t * bhat + s // P) * P + (dst[o0:o1] - gt * P)
        sel[c, s % P, col] = w[o0:o1].astype(ml_dtypes.bfloat16)
        gs = t * slots_per_tile + s
        gidx[c, gs % 16, gs // 16] = src[o0:o1].astype(np.int16)

    gidx = np.tile(gidx, (1, 8, 1))  # replicate to 128 partitions

    per_core = []
    for c in range(C):
        per_core.append(
            {
                "sel": np.ascontiguousarray(sel[c]),
                "gidx": np.ascontiguousarray(gidx[c]),
                "xT": np.ascontiguousarray(x[c * nl : (c + 1) * nl].T),
            }
        )
    meta = {"n": n, "nl": nl, "t": t_per_core, "bhat": bhat, "hid": hid,
            "feat": x.shape[1]}
    return per_core, meta


def _build(meta):
    n, nl, T, bhat, hid, feat = (
        meta["n"], meta["nl"], meta["t"], meta["bhat"], meta["hid"],
        meta["feat"],
    )
    nk = feat // P            # K chunks for conv1 (4)
    spt = bhat * P            # slots per tile
    RG = [list(range(C))]

    nc = bacc.Bacc(None, target_bir_lowering=False, num_devices=C)

    sel_d = nc.dram_tensor("sel", [P, T * spt], BF16, kind="ExternalInput")
    gidx_d = nc.dram_tensor("gidx", [P, T * spt // 16], I16, kind="ExternalInput")
    xT_d = nc.dram_tensor("xT", [feat, nl], F32, kind="ExternalInput")
    W1_d = nc.dram_tensor("W1", [feat, hid], F32, kind="ExternalInput")
    W2_d = nc.dram_tensor("W2", [hid, hid], F32, kind="ExternalInput")
    Wa1_d = nc.dram_tensor("Wa1", [hid, hid], F32, kind="ExternalInput")
    Wa2_d = nc.dram_tensor("Wa2", [hid, feat], F32, kind="ExternalInput")
    Ws1_d = nc.dram_tensor("Ws1", [hid, hid], F32, kind="ExternalInput")
    b1_d = nc.dram_tensor("b1", [1, hid], F32, kind="ExternalInput")
    b2_d = nc.dram_tensor("b2", [1, hid], F32, kind="ExternalInput")
    ba1_d = nc.dram_tensor("ba1", [1, hid], F32, kind="ExternalInput")
    ba2_d = nc.dram_tensor("ba2", [1, feat], F32, kind="ExternalInput")
    bs1_d = nc.dram_tensor("bs1", [1, hid], F32, kind="ExternalInput")

    ahat_d = nc.dram_tensor("ahat", [nl, n], F32, kind="ExternalOutput")
    xhat_d = nc.dram_tensor("xhat", [nl, feat], F32, kind="ExternalOutput")

    with tile.TileContext(nc) as tc:
        with (
            tc.tile_pool(name="const", bufs=1) as const,
            tc.tile_pool(name="work", bufs=2) as work,
            tc.tile_pool(name="psum", bufs=2, space="PSUM") as psum,
            tc.tile_pool(name="dram", bufs=1, space="DRAM") as dram,
        ):
            nc.gpsimd.load_library(library_config.mlp)

            # ---- constant loads ----
            sel_sb = const.tile([P, T * spt], BF16)
            n_sel_dma = 4
            sel_cols = T * spt // n_sel_dma
            for i in range(n_sel_dma):
                cs = slice(i * sel_cols, (i + 1) * sel_cols)
                nc.sync.dma_start(sel_sb[:, cs], sel_d[:, cs])
            gidx_sb = const.tile([P, T * spt // 16], I16)
            nc.sync.dma_start(gidx_sb[:, :], gidx_d[:, :])

            W1_sb = const.tile([P, nk * hid], BF16)
            for k in range(nk):
                nc.gpsimd.dma_start(
                    W1_sb[:, k * hid : (k + 1) * hid],
                    W1_d[k * P : (k + 1) * P, :],
                )
            W2_sb = const.tile([P, hid], BF16)
            nc.gpsimd.dma_start(W2_sb[:, :], W2_d[:, :])
            Wa1_sb = const.tile([P, hid], BF16)
            nc.gpsimd.dma_start(Wa1_sb[:, :], Wa1_d[:, :])
            Ws1_sb = const.tile([P, hid], BF16)
            nc.gpsimd.dma_start(Ws1_sb[:, :], Ws1_d[:, :])
            Wa2_sb = const.tile([P, feat], BF16)
            nc.gpsimd.dma_start(Wa2_sb[:, :], Wa2_d[:, :])
            b1_row = const.tile([1, hid], BF16)
            nc.gpsimd.dma_start(b1_row[:, :], b1_d[:, :])
            b2_row = const.tile([1, hid], BF16)
            nc.gpsimd.dma_start(b2_row[:, :], b2_d[:, :])
            ba1_row = const.tile([1, hid], BF16)
            nc.gpsimd.dma_start(ba1_row[:, :], ba1_d[:, :])
            ba2_row = const.tile([1, feat], BF16)
            nc.gpsimd.dma_start(ba2_row[:, :], ba2_d[:, :])
            bs1_row = const.tile([1, hid], BF16)
            nc.gpsimd.dma_start(bs1_row[:, :], bs1_d[:, :])

            ones_col = const.tile([P, 1], BF16)
            nc.vector.memset(ones_col[:, :], 1.0)
            ones_row = const.tile([1, P], BF16)
            nc.vector.memset(ones_row[:, :], 1.0)

            dinv_col = const.tile([P, T], F32)
            sq_col = const.tile([P, T], F32)
            sqdeg_row = const.tile([1, nl], BF16)
            xh_all = const.tile([P, T * hid], BF16)
            hT1 = const.tile([P, nl], BF16)
            hT2 = const.tile([P, nl], BF16)
            AT = const.tile([P, nl], BF16)
            sT_own = const.tile([P, nl], BF16)
            s_fm = const.tile([P, n], BF16)

            # ---- deg = Sel.T @ 1 ; dinv = 1/sqrt(deg) ----
            for t in range(T):
                dg = psum.tile([P, 1], F32, tag="ups")
                for b in range(bhat):
                    blk = t * bhat + b
                    nc.tensor.matmul(
                        dg[:, :],
                        lhsT=sel_sb[:, blk * P : (blk + 1) * P],
                        rhs=ones_col[:, :],
                        start=(b == 0),
                        stop=(b == bhat - 1),
                    )
                nc.scalar.sqrt(sq_col[:, t : t + 1], dg[:, :])
                nc.vector.reciprocal(dinv_col[:, t : t + 1], sq_col[:, t : t + 1])

            # sqrt(deg) as a [1, nl] row via a DRAM round trip
            sq_dram = dram.tile([nl], F32)
            nc.sync.dma_start(
                sq_dram[:].rearrange("(t p) -> p t", p=P), sq_col[:, :]
            )
            nc.gpsimd.dma_start(sqdeg_row[0:1, :], sq_dram[None, :])

            # ---- per-conv DRAM buffers ----
            def dram_pair(nm, width):
                g = dram.tile([nl, width], BF16, name=f"g_{nm}")
                ag = dram.tile([n, width], BF16, addr_space="Shared",
                               name=f"ag_{nm}")
                return g, ag

            g1_dram, ag1 = dram_pair("c1", hid)
            g2_dram, ag2 = dram_pair("c2", hid)
            ga1_dram, aga1 = dram_pair("a1", hid)
            gs1_dram, ags1 = dram_pair("s1", hid)
            gxh_dram, agxh = dram_pair("xh", hid)
            h1_dram = dram.tile([nl, hid], BF16)
            s_dram = dram.tile([nl, hid], BF16)
            A_dram = dram.tile([nl, hid], BF16)
            ag_s = dram.tile([n, hid], BF16, addr_space="Shared")

            def linear_g(t, lhsT_fn, nk_, rhs_fn, g_dram):
                """u = h @ W for tile t, g = dinv*u -> g_dram rows."""
                ups = psum.tile([P, hid], F32, tag="ups")
                for k in range(nk_):
                    nc.tensor.matmul(
                        ups[:, :], lhsT=lhsT_fn(t, k), rhs=rhs_fn(k),
                        start=(k == 0), stop=(k == nk_ - 1),
                    )
                g = work.tile([P, hid], BF16, tag="gtile")
                nc.vector.tensor_scalar_mul(g[:, :], ups[:, :],
                                            dinv_col[:, t : t + 1])
                nc.sync.dma_start(g_dram[t * P : (t + 1) * P, :], g[:, :])

            nidx_reg = nc.gpsimd.to_reg(GC * P)

            def aggregate(ag, bias_row, relu, out_sb_fn, out_dram):
                """P = Sel.T @ gather(ag) (+ sqrtdeg x bias); out = act(dinv*P)."""
                for t in range(T):
                    pps = psum.tile([P, hid], F32, tag="pps")
                    for gc in range(bhat // GC):
                        gb = work.tile([P, GC, hid], BF16, tag="gbuf", bufs=4)
                        o16 = (t * spt + gc * GC * P) // 16
                        nidx = GC * P
                        nc.gpsimd.dma_gather(
                            gb[:, :, :], ag[:, :],
                            gidx_sb[:, o16 : o16 + nidx // 16],
                            nidx, nidx_reg, hid,
                        )
                        for b in range(GC):
                            blk = t * bhat + gc * GC + b
                            last = (gc == bhat // GC - 1) and (b == GC - 1)
                            nc.tensor.matmul(
                                pps[:, :],
                                lhsT=sel_sb[:, blk * P : (blk + 1) * P],
                                rhs=gb[:, b, :],
                                start=(gc == 0 and b == 0),
                                stop=(last and bias_row is None),
                            )
                    if bias_row is not None:
                        nc.tensor.matmul(
                            pps[:, :],
                            lhsT=sqdeg_row[0:1, t * P : (t + 1) * P],
                            rhs=bias_row[0:1, :],
                            start=False, stop=True,
                        )
                    out_sb = out_sb_fn(t)
                    nc.scalar.activation(
                        out_sb, pps[:, :],
                        mybir.ActivationFunctionType.Relu if relu
                        else mybir.ActivationFunctionType.Copy,
                        scale=dinv_col[:, t : t + 1],
                    )
                    if out_dram is not None:
                        nc.sync.dma_start(
                            out_dram[t * P : (t + 1) * P, :], out_sb
                        )

            def ag_collective(g_dram, ag):
                nc.gpsimd.collective_compute(
                    "AllGather", mybir.AluOpType.bypass, replica_groups=RG,
                    ins=[g_dram.opt()], outs=[ag.opt()],
                )

            def work_out_tile(t):
                return work.tile([P, hid], BF16, tag="htile", name="htile")

            # ---- conv1: x @ W1 -> aggregate -> h1 ----
            with tc.tile_pool(name="xpool", bufs=1) as xpool:
                xT_sb = xpool.tile([P, nk * nl], BF16)
                for k in range(nk):
                    nc.gpsimd.dma_start(
                        xT_sb[:, k * nl : (k + 1) * nl],
                        xT_d[k * P : (k + 1) * P, :],
                    )
                for t in range(T):
                    linear_g(
                        t,
                        lambda t_, k: xT_sb[:, k * nl + t_ * P : k * nl + (t_ + 1) * P],
                        nk,
                        lambda k: W1_sb[:, k * hid : (k + 1) * hid],
                        g1_dram,
                    )
                ag_collective(g1_dram, ag1)
                aggregate(ag1, b1_row, True, work_out_tile, h1_dram)
            nc.sync.dma_start_transpose(hT1[:, :], h1_dram[:, :])

            # ---- conv2: h1 @ W2 -> aggregate -> h2 (stays as hT2) ----
            h2_dram = dram.tile([nl, hid], BF16)
            for t in range(T):
                linear_g(
                    t,
                    lambda t_, k: hT1[:, t_ * P : (t_ + 1) * P],
                    1,
                    lambda k: W2_sb[:, :],
                    g2_dram,
                )
            ag_collective(g2_dram, ag2)
            aggregate(ag2, b2_row, True, work_out_tile, h2_dram)
            nc.sync.dma_start_transpose(hT2[:, :], h2_dram[:, :])

            # ---- convs1 (structure): h2 @ Ws1 -> aggregate -> s ----
            for t in range(T):
                linear_g(
                    t,
                    lambda t_, k: hT2[:, t_ * P : (t_ + 1) * P],
                    1,
                    lambda k: Ws1_sb[:, :],
                    gs1_dram,
                )
            ag_collective(gs1_dram, ags1)
            aggregate(ags1, bs1_row, True, work_out_tile, s_dram)

            # ---- conva1: h2 @ Wa1 -> aggregate -> xh (kept in SBUF) ----
            for t in range(T):
                linear_g(
                    t,
                    lambda t_, k: hT2[:, t_ * P : (t_ + 1) * P],
                    1,
                    lambda k: Wa1_sb[:, :],
                    ga1_dram,
                )
            ag_collective(ga1_dram, aga1)
            aggregate(
                aga1, ba1_row, True,
                lambda t: xh_all[:, t * hid : (t + 1) * hid], None,
            )

            # ---- a_hat = s_own @ s_full.T ----
            ag_collective(s_dram, ag_s)
            nc.sync.dma_start_transpose(sT_own[:, :], s_dram[:, :])
            for j in range(4):
                rs = slice(j * (n // 4), (j + 1) * (n // 4))
                nc.sync.dma_start_transpose(s_fm[:, rs], ag_s[rs, :])
            NJ = n // 512
            GJ = 4 if NJ % 4 == 0 else (2 if NJ % 2 == 0 else 1)
            for t in range(T):
                for j4 in range(NJ // GJ):
                    stg = work.tile([P, 512 * GJ], F32, tag="astg")
                    for jj in range(GJ):
                        j = j4 * GJ + jj
                        aps = psum.tile([P, 512], F32, tag="aps")
                        nc.tensor.matmul(
                            aps[:, :],
                            lhsT=sT_own[:, t * P : (t + 1) * P],
                            rhs=s_fm[:, j * 512 : (j + 1) * 512],
                            start=True, stop=True,
                        )
                        copy_eng = nc.vector.tensor_copy if jj % 2 else nc.scalar.copy
                        copy_eng(stg[:, jj * 512 : (jj + 1) * 512], aps[:, :])
                    nc.sync.dma_start(
                        ahat_d[t * P : (t + 1) * P,
                               j4 * 512 * GJ : (j4 + 1) * 512 * GJ],
                        stg[:, :],
                    )

            # ---- conva2: aggregate(dinv*xh) -> A; x_hat = relu(A@Wa2+ba2) ----
            for t in range(T):
                g = work.tile([P, hid], BF16, tag="gtile")
                nc.vector.tensor_scalar_mul(
                    g[:, :], xh_all[:, t * hid : (t + 1) * hid],
                    dinv_col[:, t : t + 1],
                )
                nc.sync.dma_start(gxh_dram[t * P : (t + 1) * P, :], g[:, :])
            ag_collective(gxh_dram, agxh)
            aggregate(agxh, None, False, work_out_tile, A_dram)
            nc.sync.dma_start_transpose(AT[:, :], A_dram[:, :])
            for t in range(T):
                xps = psum.tile([P, feat], F32, tag="xps", bufs=1)
                nc.tensor.matmul(
                    xps[:, :], lhsT=AT[:, t * P : (t + 1) * P],
                    rhs=Wa2_sb[:, :], start=True, stop=False,
                )
                nc.tensor.matmul(
                    xps[:, :], lhsT=ones_row[0:1, :], rhs=ba2_row[0:1, :],
                    start=False, stop=True,
                )
                xo = work.tile([P, feat], F32, tag="xo")
                nc.scalar.activation(
                    xo[:, :], xps[:, :], mybir.ActivationFunctionType.Relu
                )
                nc.sync.dma_start(xhat_d[t * P : (t + 1) * P, :], xo[:, :])

    nc.compile()
    return nc


def _make_in_maps(inputs, per_core, meta):
    n, nl = meta["n"], meta["nl"]
    shared = {
        "W1": np.asarray(inputs["W1"], np.float32),
        "W2": np.asarray(inputs["W2"], np.float32),
        "Wa1": np.asarray(inputs["Wa1"], np.float32),
        "Wa2": np.asarray(inputs["Wa2"], np.float32),
        "Ws1": np.asarray(inputs["Ws1"], np.float32),
        "b1": np.asarray(inputs["b1"], np.float32).reshape(1, -1),
        "b2": np.asarray(inputs["b2"], np.float32).reshape(1, -1),
        "ba1": np.asarray(inputs["ba1"], np.float32).reshape(1, -1),
        "ba2": np.asarray(inputs["ba2"], np.float32).reshape(1, -1),
        "bs1": np.asarray(inputs["bs1"], np.float32).reshape(1, -1),
    }
    in_maps = []
    for c in range(C):
        m = dict(per_core[c])
        m.update(shared)
        in_maps.append(m)
    return in_maps


def run(inputs, trace=False):
    """Build + run on 8 cores. Returns ((a_hat, x_hat), BassKernelResults)."""
    x = np.asarray(inputs["x"], np.float32)
    n, feat = x.shape
    per_core, meta = _prep(
        x, np.asarray(inputs["edge_index"]), np.asarray(inputs["edge_weight"]),
        n, np.asarray(inputs["W1"]).shape[1],
    )
    nc = _build(meta)
    in_maps = _make_in_maps(inputs, per_core, meta)
    res = run_bass_kernel_spmd(
        nc, in_maps, core_ids=list(range(C)), trace=trace,
    )
    ahat = np.concatenate([res.results[c]["ahat"] for c in range(C)], axis=0)
    xhat = np.concatenate([res.results[c]["xhat"] for c in range(C)], axis=0)
    return (ahat, xhat), res


def kernel(**inputs):
    (ahat, xhat), _ = run(inputs)
    return ahat, xhat
